# revision 4
# baseline (speedup 1.0000x reference)
"""GQA attention decode kernel for Trainium2 (Bass/Tile), SPMD over 8 NeuronCores.

Sharding: kv-head axis (K=2) x batch groups (4) -> 8 cores.
Core c: kv head k=c%2, batches [2*(c//2), 2*(c//2)+2).
Each core computes q/k/v projections + RoPE for its head group, attends over
its shard of the KV cache (only rows [0, cur_ind+T) ever contribute), and
produces a partial output projection. Host sums the two kv-head partials.

v2: all matmul operands in bf16 (4x PE rate, half the HBM bytes; tolerance is
rel 2e-2 so bf16 noise ~3e-3 is fine). K cache is pre-transposed on the host
to [H, s] layout so the hot loop needs no PE transposes; exp runs once per
512-row chunk instead of per 128 rows.

Shapes (hardcoded from the problem spec):
  x [8,16,1536], k_cache/v_cache [8,8192,2,128],
  wq [1536,12,128], wk/wv [1536,2,128], wo [12,128,1536], out [8,16,1536]
"""

import sys

if "/opt/trn_rl_repo" not in sys.path:
    sys.path.insert(0, "/opt/trn_rl_repo")

import numpy as np
import ml_dtypes

BF = ml_dtypes.bfloat16

B, T, S, D, N, K, H = 8, 16, 8192, 1536, 12, 2, 128
G = N // K            # 6 q heads per kv head
BG = 4                # batch groups
BL = B // BG          # 2 local batches per core
CS = 512              # cache s-chunk
DC = D // 128         # 12 contraction chunks
GH = G * H            # 768
GT = G * T            # 96
ROPE_THETA = 1000000.0
NEG = -1.0e30

_built = {}


# ---------------------------------------------------------------- host math
def _host_rope(positions):
    # positions [b, t] int32 -> sin, cos [b, t, 64] float32 (mirrors reference)
    frac = np.arange(0, H, 2, dtype=np.float32) / np.float32(H)
    timescale = np.power(np.float32(ROPE_THETA), frac, dtype=np.float32)
    ang = positions[..., None].astype(np.float32) / timescale
    return np.sin(ang, dtype=np.float32), np.cos(ang, dtype=np.float32)


def _host_mask(segment_ids, start_ind, cur):
    seg = np.asarray(segment_ids, np.int32)
    sti = np.asarray(start_ind, np.int32)
    nonpad = seg != 0
    left_pads = np.argmax(nonpad, axis=-1).astype(np.int32)
    start = np.where(sti < 0, left_pads, sti).astype(np.int32)
    positions = np.maximum(np.cumsum(nonpad.astype(np.int32), axis=-1) - 1, 0) + cur

    q_pos = cur + np.arange(T, dtype=np.int32)[None, :] - start[:, None]
    ts_ = np.arange(S, dtype=np.int32)
    kv_seg = (ts_[None, :] >= start[:, None]) & (ts_[None, :] < cur + T)
    k_pos = ts_[None, :] - start[:, None]
    causal = k_pos[:, None, :] <= q_pos[:, :, None]
    segm = kv_seg[:, None, :].astype(np.int32) == seg[:, :, None]
    mask = causal & segm  # [b, t, S] True = attend
    return mask, positions


def _numpy_reference(x, k_cache, v_cache, wq, bq, wk, bk, wv, bv, wo,
                     segment_ids, start_ind, cur):
    # Full-precision numpy fallback (used only for inputs outside the
    # spec envelope: non-zero biases, odd cur_ind alignment, pad tokens).
    mask, positions = _host_mask(segment_ids, start_ind, cur)
    sin, cos = _host_rope(positions)

    def rope(t):  # t [b,tk,n,h]
        h2 = H // 2
        x1, x2 = t[..., :h2], t[..., h2:]
        s = sin[:, :, None, :]
        c = cos[:, :, None, :]
        return np.concatenate([x1 * c - x2 * s, x2 * c + x1 * s], axis=-1)

    q = np.einsum("btd,dnh->btnh", x, wq) + bq
    kp = np.einsum("btd,dkh->btkh", x, wk) + bk
    v = np.einsum("btd,dkh->btkh", x, wv) + bv
    q = rope(q)
    kp = rope(kp)
    kc = np.array(k_cache)
    vc = np.array(v_cache)
    kc[:, cur:cur + T] = kp
    vc[:, cur:cur + T] = v
    scale = np.float32(H) ** -0.5
    qg = q.reshape(B, T, K, G, H)
    logits = np.einsum("btkgh,bskh->btskg", qg, kc) * scale
    logits = np.where(mask[:, :, :, None, None], logits, np.float32(-3.3895314e38))
    logits = logits - logits.max(axis=2, keepdims=True)
    w = np.exp(logits.astype(np.float32))
    w = w / w.sum(axis=2, keepdims=True)
    qkv = np.einsum("btskg,bskh->btkgh", w, vc).reshape(B, T, N, H)
    return np.einsum("btnh,nhd->btd", qkv, wo).astype(np.float32)


# ---------------------------------------------------------------- device build
def _build(sold):
    import concourse.bass as bass
    import concourse.bacc as bacc
    import concourse.tile as tile
    from concourse import mybir
    from concourse.masks import make_identity

    f32 = mybir.dt.float32
    bf16 = mybir.dt.bfloat16
    nch = sold // CS
    BT = BL * T  # 32

    nc = bacc.Bacc(None)
    xt_d = nc.declare_dram_parameter("xt", [128, DC, BT], bf16, isOutput=False)
    wpk_d = nc.declare_dram_parameter("wpk", [DC, 128, GH + H], bf16, isOutput=False)
    wvk_d = nc.declare_dram_parameter("wvk", [DC, 128, H], bf16, isOutput=False)
    wok_d = nc.declare_dram_parameter("wok", [G, H, D], bf16, isOutput=False)
    # k cache pre-transposed on host: ktp[lb, h, ch, j, p] = K[lb, ch*512+4p+j, h]
    # so block j of a chunk pairs with the (p j)-rearranged v rows directly.
    ktp_d = nc.declare_dram_parameter("ktp", [BL, 128, nch, 4, 128], bf16, isOutput=False)
    vcp_d = nc.declare_dram_parameter("vcp", [BL, sold, H + 1], bf16, isOutput=False)
    ropeq_d = nc.declare_dram_parameter("ropeq", [128, 2, G, BT], f32, isOutput=False)
    ropek_d = nc.declare_dram_parameter("ropek", [128, 2, BT], f32, isOutput=False)
    nmask_d = nc.declare_dram_parameter("nmask", [T, BL, GT], f32, isOutput=False)
    outp_d = nc.declare_dram_parameter("out", [BT, D], f32, isOutput=True)

    with tile.TileContext(nc) as tc:
        with (
            tc.tile_pool(name="cpool", bufs=1) as cpool,
            tc.tile_pool(name="kvpool", bufs=4) as kvp,
            tc.tile_pool(name="wtpool", bufs=3) as wtp,
            tc.tile_pool(name="spool", bufs=2) as sp,
            tc.tile_pool(name="pt", bufs=2, space="PSUM") as pt,
            tc.tile_pool(name="pl", bufs=2, space="PSUM") as pl,
            tc.tile_pool(name="pacc", bufs=1, space="PSUM") as pacc,
            tc.tile_pool(name="pp", bufs=2, space="PSUM") as pp,
        ):
            identf = cpool.tile([128, 128], f32)
            make_identity(nc, identf)
            identb = cpool.tile([GT, GT], bf16)
            make_identity(nc, identb)

            # ---- constant loads; wpk gates the critical path, wok is needed last
            x_t = cpool.tile([128, DC, BT], bf16)
            nc.sync.dma_start(out=x_t, in_=xt_d[:])
            rq_t = cpool.tile([128, 2, G, BT], f32)
            nc.sync.dma_start(out=rq_t, in_=ropeq_d[:])
            rk_t = cpool.tile([128, 2, BT], f32)
            nc.sync.dma_start(out=rk_t, in_=ropek_d[:])
            nm_t = cpool.tile([T, BL, GT], f32)
            nc.sync.dma_start(out=nm_t, in_=nmask_d[:])

            wpk_t = cpool.tile([128, DC, GH + H], bf16)
            wv_t = cpool.tile([128, DC, H], bf16)
            for c in range(DC):
                nc.sync.dma_start(out=wpk_t[:, c, :], in_=wpk_d[c])
                nc.sync.dma_start(out=wv_t[:, c, :], in_=wvk_d[c])
            wo_t = cpool.tile([128, G, D], bf16)
            for g in range(G):
                nc.sync.dma_start(out=wo_t[:, g, :], in_=wok_d[g])

            # ---- q/k projections (x chunks stationary, packed [wq|wk] moving)
            qp0 = pp.tile([BT, 512], f32, tag="pp")
            qp1 = pp.tile([BT, 384], f32, tag="pp")
            for c in range(DC):
                st, spf = (c == 0), (c == DC - 1)
                nc.tensor.matmul(qp0, x_t[:, c, :], wpk_t[:, c, 0:512], start=st, stop=spf)
                nc.tensor.matmul(qp1, x_t[:, c, :], wpk_t[:, c, 512:896], start=st, stop=spf)
            q_sb = cpool.tile([BT, GH], f32)
            nc.vector.tensor_copy(q_sb[:, 0:512], qp0)
            nc.vector.tensor_copy(q_sb[:, 512:768], qp1[:, 0:256])
            k_sb = cpool.tile([BT, H], f32)
            nc.vector.tensor_copy(k_sb, qp1[:, 256:384])

            vN = cpool.tile([T, BL, H + 1], bf16)
            for lb in range(BL):
                vp = pp.tile([T, H], f32, tag="pp", name=f"vp{lb}")
                for c in range(DC):
                    nc.tensor.matmul(vp, x_t[:, c, lb * T:(lb + 1) * T], wv_t[:, c, :],
                                     start=(c == 0), stop=(c == DC - 1))
                nc.vector.tensor_copy(vN[:, lb, 0:H], vp)
            nc.vector.memset(vN[:, :, H:H + 1], 1.0)

            # ---- q -> qT (f32), RoPE (scale folded in coeffs), cast bf16
            qTr = cpool.tile([128, G, BT], f32)
            qSw = cpool.tile([128, G, BT], f32)
            for g in range(G):
                tp = pt.tile([128, BT], f32, tag="pt")
                nc.tensor.transpose(tp, q_sb[:, g * H:(g + 1) * H], identf[:BT, :BT])
                nc.vector.tensor_copy(qTr[:, g, :], tp)
            # rotated halves via SBUF->SBUF DMA (cross-partition move)
            nc.sync.dma_start(out=qSw[0:64, :, :], in_=qTr[64:128, :, :])
            nc.sync.dma_start(out=qSw[64:128, :, :], in_=qTr[0:64, :, :])
            qtmp = cpool.tile([128, G, BT], f32)
            qRf = cpool.tile([128, G, BT], f32)
            qR = cpool.tile([128, G, BT], bf16)
            nc.vector.tensor_tensor(qtmp, qTr, rq_t[:, 0], mybir.AluOpType.mult)
            nc.vector.tensor_tensor(qRf, qSw, rq_t[:, 1], mybir.AluOpType.mult)
            nc.vector.tensor_tensor(qR, qRf, qtmp, mybir.AluOpType.add)

            # ---- k_new -> kT (f32), RoPE (no scale), cast bf16
            kTr = cpool.tile([128, BT], f32)
            kSw = cpool.tile([128, BT], f32)
            tpk = pt.tile([128, BT], f32, tag="pt")
            nc.tensor.transpose(tpk, k_sb, identf[:BT, :BT])
            nc.vector.tensor_copy(kTr, tpk)
            nc.sync.dma_start(out=kSw[0:64, :], in_=kTr[64:128, :])
            nc.sync.dma_start(out=kSw[64:128, :], in_=kTr[0:64, :])
            ktmp = cpool.tile([128, BT], f32)
            kRf = cpool.tile([128, BT], f32)
            kR = cpool.tile([128, BT], bf16)
            nc.vector.tensor_tensor(ktmp, kTr, rk_t[:, 0], mybir.AluOpType.mult)
            nc.vector.tensor_tensor(kRf, kSw, rk_t[:, 1], mybir.AluOpType.mult)
            nc.vector.tensor_tensor(kR, kRf, ktmp, mybir.AluOpType.add)

            # ---- new-token logits + masked exp
            wTns = []
            for lb in range(BL):
                lpn = pl.tile([T, GT], f32, tag="pl", name=f"lpn{lb}")
                nc.tensor.matmul(lpn, kR[:, lb * T:(lb + 1) * T],
                                 qR[:, :, lb * T:(lb + 1) * T], start=True, stop=True)
                nc.vector.tensor_tensor(lpn, lpn, nm_t[:, lb, :], mybir.AluOpType.add)
                wTn = sp.tile([T, GT], bf16, tag="wtn", name=f"wTn{lb}")
                nc.scalar.activation(wTn, lpn, mybir.ActivationFunctionType.Exp)
                wTns.append(wTn)

            # ---- attention over the cache (no transposes: K comes in [h, s])
            qkvPs = []
            for lb in range(BL):
                qkvP = pacc.tile([GT, H + 1], f32, tag="pacc", name=f"qkvP{lb}")
                qkvPs.append(qkvP)
                for ch in range(nch):
                    kct = kvp.tile([128, 4, 128], bf16, tag="kc")
                    nc.gpsimd.dma_start(out=kct, in_=ktp_d[lb, :, ch])
                    vct = kvp.tile([128, 4, H + 1], bf16, tag="vc")
                    nc.gpsimd.dma_start(
                        out=vct,
                        in_=vcp_d[lb, ch * CS:(ch + 1) * CS, :].rearrange(
                            "(p j) h -> p j h", p=128))
                    lps = pl.tile([128, 4, GT], f32, tag="pl")
                    for j in range(4):
                        nc.tensor.matmul(lps[:, j, :], kct[:, j, :],
                                         qR[:, :, lb * T:(lb + 1) * T],
                                         start=True, stop=True)
                    wT = wtp.tile([128, 4, GT], bf16, tag="wt")
                    nc.scalar.activation(wT, lps, mybir.ActivationFunctionType.Exp)
                    for j in range(4):
                        nc.tensor.matmul(qkvP, wT[:, j, :], vct[:, j, :],
                                         start=(ch == 0 and j == 0), stop=False,
                                         skip_group_check=True)
                # new tokens (kv rows [cur, cur+T) live on-chip)
                nc.tensor.matmul(qkvP, wTns[lb], vN[:, lb, :], start=False, stop=True,
                                 skip_group_check=True)

            # ---- normalize + transpose qkv (epilogues after both batches)
            qkvT = cpool.tile([128, G, BT], bf16)
            for lb in range(BL):
                qkvP = qkvPs[lb]
                rec = sp.tile([GT, 1], f32, tag="rec")
                nc.vector.reciprocal(rec, qkvP[:, H:H + 1])
                qkvN = sp.tile([GT, H], bf16, tag="qkvN")
                nc.vector.tensor_scalar_mul(qkvN, qkvP[:, 0:H], rec)
                tp3 = pt.tile([128, GT], bf16, tag="pt")
                nc.tensor.transpose(tp3, qkvN, identb)
                nc.vector.tensor_copy(
                    qkvT[:, :, lb * T:(lb + 1) * T],
                    tp3.rearrange("h (g t) -> h g t", g=G))

            # ---- output projection: out[bt, d] = sum_g qkvT[h,g,bt]^T wo[h,g,d]
            for db in range(3):
                oP = pp.tile([BT, 512], f32, tag="pp")
                for g in range(G):
                    nc.tensor.matmul(oP, qkvT[:, g, :], wo_t[:, g, db * 512:(db + 1) * 512],
                                     start=(g == 0), stop=(g == G - 1))
                o_sb = sp.tile([BT, 512], f32, tag="osb")
                nc.vector.tensor_copy(o_sb, oP)
                nc.sync.dma_start(out=outp_d[:, db * 512:(db + 1) * 512], in_=o_sb)

    nc.compile()  # bacc passes: splits multi-wait instructions (TRN2 allows 1)
    return nc


# ---------------------------------------------------------------- entry point
def kernel(x, k_cache, v_cache, wq, bq, wk, bk, wv, bv, wo,
           segment_ids, start_ind, cur_ind):
    x = np.asarray(x, np.float32)
    k_cache = np.asarray(k_cache, np.float32)
    v_cache = np.asarray(v_cache, np.float32)
    wq = np.asarray(wq, np.float32)
    wk = np.asarray(wk, np.float32)
    wv = np.asarray(wv, np.float32)
    wo = np.asarray(wo, np.float32)
    cur = int(np.asarray(cur_ind))

    mask, positions = _host_mask(segment_ids, start_ind, cur)

    spec_ok = (
        cur % CS == 0 and 0 < cur and cur + T <= S
        and not np.any(np.asarray(bq)) and not np.any(np.asarray(bk))
        and not np.any(np.asarray(bv))
        and not np.any(mask[:, :, cur + T:])          # nothing attended past new rows
        and bool(np.all(np.any(mask, axis=2)))        # no fully-masked query row
        and bool(np.all(mask[:, :, :cur]))            # all old-cache rows attended
    )
    if not spec_ok:
        return _numpy_reference(x, k_cache, v_cache, wq, bq, wk, bk, wv, bv, wo,
                                segment_ids, start_ind, cur)

    sold = cur
    key = sold
    if key not in _built:
        _built[key] = _build(sold)
    nc = _built[key]

    inputs = dict(x=x, k_cache=k_cache, v_cache=v_cache, wq=wq, wk=wk, wv=wv,
                  wo=wo, segment_ids=segment_ids, start_ind=start_ind,
                  cur_ind=cur)
    in_maps = _make_in_maps(inputs, sold, mask=mask, positions=positions)

    global _last_in_maps
    _last_in_maps = in_maps

    import os
    from concourse.bass_utils import run_bass_kernel_spmd
    trace = os.environ.get("KERNEL_TRACE", "0") == "1"
    res = run_bass_kernel_spmd(nc, in_maps, core_ids=list(range(8)), trace=trace)
    if trace and res.exec_time_ns is not None:
        print(f"HW exec time: {res.exec_time_ns} ns")

    out = np.zeros((B, T, D), np.float32)
    for c in range(8):
        bg = c // 2
        out[bg * BL:(bg + 1) * BL] += res.results[c]["out"].reshape(BL, T, D)
    return out


def _make_in_maps(inputs, sold, mask=None, positions=None):
    x = np.asarray(inputs["x"], np.float32)
    k_cache = np.asarray(inputs["k_cache"], np.float32)
    v_cache = np.asarray(inputs["v_cache"], np.float32)
    wq = np.asarray(inputs["wq"], np.float32)
    wk = np.asarray(inputs["wk"], np.float32)
    wv = np.asarray(inputs["wv"], np.float32)
    wo = np.asarray(inputs["wo"], np.float32)
    cur = int(np.asarray(inputs["cur_ind"]))
    if mask is None:
        mask, positions = _host_mask(inputs["segment_ids"], inputs["start_ind"], cur)

    sin, cos = _host_rope(positions)  # [b, t, 64]
    scale = np.float32(H ** -0.5)
    nch = sold // CS
    BT = BL * T

    # rope coeff layouts: rows h<64 -> (cos, -sin); h>=64 -> (cos, +sin)
    def rope_pack(bsl, ncols_g, with_scale):
        # returns [128, 2, ncols_g, BL*T]
        cs = cos[bsl]  # [BL, T, 64]
        sn = sin[bsl]
        ccol = np.transpose(cs, (2, 0, 1)).reshape(64, BL * T)  # [64, (b,t)]
        scol = np.transpose(sn, (2, 0, 1)).reshape(64, BL * T)
        top_c, bot_c = ccol, ccol
        top_s, bot_s = -scol, scol
        c128 = np.concatenate([top_c, bot_c], axis=0)   # [128, BT]
        s128 = np.concatenate([top_s, bot_s], axis=0)
        if with_scale:
            c128 = c128 * scale
            s128 = s128 * scale
        pack = np.stack([c128, s128], axis=1)           # [128, 2, BT]
        pack = np.repeat(pack[:, :, None, :], ncols_g, axis=2)
        return np.ascontiguousarray(pack, np.float32)

    in_maps = []
    for c in range(8):
        k = c % 2
        bg = c // 2
        bsl = slice(bg * BL, (bg + 1) * BL)
        wq4 = wq.reshape(DC, 128, N, H)[:, :, k * G:(k + 1) * G, :].reshape(DC, 128, GH)
        wk4 = wk.reshape(DC, 128, K, H)[:, :, k, :]
        wpk = np.concatenate([wq4, wk4], axis=2).astype(BF)   # [DC, 128, 896]
        wv4 = wv.reshape(DC, 128, K, H)[:, :, k, :].astype(BF)
        kc = k_cache[bsl, :sold, k, :]                        # [BL, sold, H]
        # ktp[lb, h, ch, j, p] = K[lb, ch*512 + 4p + j, h]
        ktp = np.ascontiguousarray(
            kc.reshape(BL, nch, 128, 4, H).transpose(0, 4, 1, 3, 2).astype(BF))
        vcs = np.concatenate(
            [v_cache[bsl, :sold, k, :], np.ones((BL, sold, 1), np.float32)],
            axis=-1).astype(BF)
        xr = np.ascontiguousarray(
            x[bsl].reshape(BT, DC, 128).transpose(2, 1, 0).astype(BF))
        # additive mask for the new-token block: [T(s_new), BL, G*T]
        nm = np.where(mask[bsl][:, :, cur:cur + T], np.float32(0), np.float32(NEG))
        nm = np.transpose(nm, (2, 0, 1))                 # [s_new, BL, t]
        nm = np.repeat(nm[:, :, None, :], G, axis=2).reshape(T, BL, GT)
        in_maps.append({
            "xt": xr,
            "wpk": np.ascontiguousarray(wpk),
            "wvk": np.ascontiguousarray(wv4),
            "wok": np.ascontiguousarray(wo[k * G:(k + 1) * G].astype(BF)),
            "ktp": ktp,
            "vcp": np.ascontiguousarray(vcs),
            "ropeq": rope_pack(bsl, G, True),
            "ropek": rope_pack(bsl, 1, False).reshape(128, 2, BT),
            "nmask": np.ascontiguousarray(nm, np.float32),
        })

    return in_maps


# revision 20
# speedup vs baseline: 1.1751x; 1.1751x over previous
"""GQA attention decode kernel for Trainium2 (Bass/Tile), SPMD over 8 NeuronCores.

Sharding: kv-head axis (K=2) x batch groups (4) -> 8 cores.
Core c: kv head k=c%2, batches [2*(c//2), 2*(c//2)+2).
Each core computes q/k/v projections + RoPE for its head group, attends over
its shard of the KV cache (only rows [0, cur_ind+T) ever contribute), and
produces a partial output projection. Host sums the two kv-head partials.

v3: all matmul operands bf16 (tolerance is rel 2e-2; bf16 lands ~3.5e-3).
K cache is pre-transposed on the host to [h, s-block] layout so the hot loop
needs no PE transposes, and every DRAM tensor is host-packed so each one
loads with a single large contiguous-per-partition DMA (dma_start issue is
~0.6us each and serializes per engine -- v2 spent >40us just issuing DMAs).
A PE warmup train keeps the HAM clock-gate at 2.4 GHz, and the hot loop is
software-pipelined (logits of chunk n+2 issue before qkv of chunk n) so the
PE never stalls on the Exp activation.

Shapes (hardcoded from the problem spec):
  x [8,16,1536], k_cache/v_cache [8,8192,2,128],
  wq [1536,12,128], wk/wv [1536,2,128], wo [12,128,1536], out [8,16,1536]
"""

import sys

if "/opt/trn_rl_repo" not in sys.path:
    sys.path.insert(0, "/opt/trn_rl_repo")

import numpy as np
import ml_dtypes

BF = ml_dtypes.bfloat16

B, T, S, D, N, K, H = 8, 16, 8192, 1536, 12, 2, 128
G = N // K            # 6 q heads per kv head
BG = 4                # batch groups
BL = B // BG          # 2 local batches per core
DC = D // 128         # 12 contraction chunks
GH = G * H            # 768
GT = G * T            # 96
ROPE_THETA = 1000000.0
NEG = -1.0e30

_built = {}


# ---------------------------------------------------------------- host math
def _host_rope(positions):
    # positions [b, t] int32 -> sin, cos [b, t, 64] float32 (mirrors reference)
    frac = np.arange(0, H, 2, dtype=np.float32) / np.float32(H)
    timescale = np.power(np.float32(ROPE_THETA), frac, dtype=np.float32)
    ang = positions[..., None].astype(np.float32) / timescale
    return np.sin(ang, dtype=np.float32), np.cos(ang, dtype=np.float32)


def _host_mask(segment_ids, start_ind, cur):
    seg = np.asarray(segment_ids, np.int32)
    sti = np.asarray(start_ind, np.int32)
    nonpad = seg != 0
    left_pads = np.argmax(nonpad, axis=-1).astype(np.int32)
    start = np.where(sti < 0, left_pads, sti).astype(np.int32)
    positions = np.maximum(np.cumsum(nonpad.astype(np.int32), axis=-1) - 1, 0) + cur

    q_pos = cur + np.arange(T, dtype=np.int32)[None, :] - start[:, None]
    ts_ = np.arange(S, dtype=np.int32)
    kv_seg = (ts_[None, :] >= start[:, None]) & (ts_[None, :] < cur + T)
    k_pos = ts_[None, :] - start[:, None]
    causal = k_pos[:, None, :] <= q_pos[:, :, None]
    segm = kv_seg[:, None, :].astype(np.int32) == seg[:, :, None]
    mask = causal & segm  # [b, t, S] True = attend
    return mask, positions


def _numpy_reference(x, k_cache, v_cache, wq, bq, wk, bk, wv, bv, wo,
                     segment_ids, start_ind, cur):
    # Full-precision numpy fallback (used only for inputs outside the
    # spec envelope: non-zero biases, odd cur_ind alignment, pad tokens).
    mask, positions = _host_mask(segment_ids, start_ind, cur)
    sin, cos = _host_rope(positions)

    def rope(t):  # t [b,tk,n,h]
        h2 = H // 2
        x1, x2 = t[..., :h2], t[..., h2:]
        s = sin[:, :, None, :]
        c = cos[:, :, None, :]
        return np.concatenate([x1 * c - x2 * s, x2 * c + x1 * s], axis=-1)

    q = np.einsum("btd,dnh->btnh", x, wq) + bq
    kp = np.einsum("btd,dkh->btkh", x, wk) + bk
    v = np.einsum("btd,dkh->btkh", x, wv) + bv
    q = rope(q)
    kp = rope(kp)
    kc = np.array(k_cache)
    vc = np.array(v_cache)
    kc[:, cur:cur + T] = kp
    vc[:, cur:cur + T] = v
    scale = np.float32(H) ** -0.5
    qg = q.reshape(B, T, K, G, H)
    logits = np.einsum("btkgh,bskh->btskg", qg, kc) * scale
    logits = np.where(mask[:, :, :, None, None], logits, np.float32(-3.3895314e38))
    logits = logits - logits.max(axis=2, keepdims=True)
    w = np.exp(logits.astype(np.float32))
    w = w / w.sum(axis=2, keepdims=True)
    qkv = np.einsum("btskg,bskh->btkgh", w, vc).reshape(B, T, N, H)
    return np.einsum("btnh,nhd->btd", qkv, wo).astype(np.float32)


# ---------------------------------------------------------------- device build
def _build(sold):
    import concourse.bass as bass
    import concourse.bacc as bacc
    import concourse.tile as tile
    from concourse import mybir
    from concourse.masks import make_identity

    f32 = mybir.dt.float32
    bf16 = mybir.dt.bfloat16
    NJ = sold // 128      # 32 s-blocks of 128 per local batch
    NG = NJ // 4          # 8 pipeline groups of 4 blocks
    BT = BL * T           # 32

    nc = bacc.Bacc(None)
    xt_d = nc.declare_dram_parameter("xt", [128, DC, BT], bf16, isOutput=False)
    # packed projection weights: per d-chunk columns = [wq 768 | wk 128 | wv 128]
    wpk_d = nc.declare_dram_parameter("wpk", [128, DC, 1024], bf16, isOutput=False)
    wok_d = nc.declare_dram_parameter("wok", [128, G, D], bf16, isOutput=False)
    # k cache pre-transposed on host: ktp[lb, h, jj, p] = K[lb, p*NJ+jj, h]
    # (matches the v layout below: vcp[lb, p, jj, :] = V[lb, p*NJ+jj, :])
    ktp_d = nc.declare_dram_parameter("ktp", [BL, 128, NJ, 128], bf16, isOutput=False)
    vcp_d = nc.declare_dram_parameter("vcp", [BL, 128, NJ, H + 1], bf16, isOutput=False)
    ropeq_d = nc.declare_dram_parameter("ropeq", [128, 2, BL, G, T], f32, isOutput=False)
    ropek_d = nc.declare_dram_parameter("ropek", [128, 2, BT], f32, isOutput=False)
    # additive mask for the fused new-token block, rows=(lb_s,t_s), cols=(lb_q,g,t_q)
    nmask_d = nc.declare_dram_parameter("nmask", [BT, BL, G, T], f32, isOutput=False)
    outp_d = nc.declare_dram_parameter("out", [BT, D], f32, isOutput=True)

    with tile.TileContext(nc) as tc:
        with (
            tc.tile_pool(name="cpool", bufs=1) as cpool,
            tc.tile_pool(name="kvpool", bufs=2) as kvp,
            tc.tile_pool(name="wtpool", bufs=3) as wtp,
            tc.tile_pool(name="spool", bufs=2) as sp,
            tc.tile_pool(name="pt", bufs=2, space="PSUM") as pt,
            tc.tile_pool(name="pl", bufs=3, space="PSUM") as pl,
            tc.tile_pool(name="pacc", bufs=1, space="PSUM") as pacc,
            tc.tile_pool(name="pp", bufs=2, space="PSUM") as pp,
        ):
            identf = cpool.tile([128, 128], f32)
            make_identity(nc, identf)
            identb = cpool.tile([GT, GT], bf16)
            make_identity(nc, identb)

            # ---- PE warmup: ~2.5us of dummy transposes so the HAM clock-gate
            # reaches 2.4 GHz before the projections (DMA-gated) begin.
            for i in range(22):
                tw = pt.tile([32, 128], f32, tag="pt")
                nc.tensor.transpose(tw, identf[:, 0:32], identf)

            # ---- constant loads; wpk gates the critical path, wok is needed last
            x_t = cpool.tile([128, DC, BT], bf16)
            nc.sync.dma_start(out=x_t, in_=xt_d[:])
            wpk_t = cpool.tile([128, DC, 1024], bf16)
            for c4 in range(4):  # 4 sub-loads of 3 d-chunks, pipelined with matmuls
                nc.sync.dma_start(out=wpk_t[:, 3 * c4:3 * (c4 + 1), :],
                                  in_=wpk_d[:, 3 * c4:3 * (c4 + 1), :])
            rq_t = cpool.tile([128, 2, BL, G, T], f32)
            nc.sync.dma_start(out=rq_t, in_=ropeq_d[:])
            rk_t = cpool.tile([128, 2, BT], f32)
            nc.sync.dma_start(out=rk_t, in_=ropek_d[:])
            nm_t = cpool.tile([BT, BL, G, T], f32)
            nc.sync.dma_start(out=nm_t, in_=nmask_d[:])
            wo_t = cpool.tile([128, G, D], bf16)
            nc.sync.dma_start(out=wo_t, in_=wok_d[:])

            # ---- cache loads: one contiguous DMA per tensor per local batch
            kcts, vcts = [], []
            for lb in range(BL):
                kct = kvp.tile([128, NJ, 128], bf16, tag="kc")
                nc.gpsimd.dma_start(out=kct, in_=ktp_d[lb])
                vct = kvp.tile([128, NJ, H + 1], bf16, tag="vc")
                nc.gpsimd.dma_start(out=vct, in_=vcp_d[lb])
                kcts.append(kct)
                vcts.append(vct)

            # ---- q/k/v projections (x chunks stationary, packed weights moving)
            qp0 = pp.tile([BT, 512], f32, tag="pp")
            qp1 = pp.tile([BT, 512], f32, tag="pp")
            for c in range(DC):
                st, spf = (c == 0), (c == DC - 1)
                nc.tensor.matmul(qp0, x_t[:, c, :], wpk_t[:, c, 0:512], start=st, stop=spf)
                nc.tensor.matmul(qp1, x_t[:, c, :], wpk_t[:, c, 512:1024], start=st, stop=spf)
            q_sb = cpool.tile([BT, GH], f32)
            nc.vector.tensor_copy(q_sb[:, 0:512], qp0)
            nc.vector.tensor_copy(q_sb[:, 512:768], qp1[:, 0:256])
            vN = cpool.tile([BT, H + 1], bf16)
            nc.vector.tensor_copy(vN[:, 0:H], qp1[:, 384:512])
            nc.vector.memset(vN[:, H:H + 1], 1.0)
            # half-swapped (rotate-half) copies, swapped along the free dim so
            # the PE transposes below stay at PSUM partition 0
            qp0v = qp0.rearrange("bt (g u h2) -> bt g u h2", g=4, u=2)
            qp1v = qp1.rearrange("bt (m u h2) -> bt m u h2", m=4, u=2)
            q_swp = cpool.tile([BT, GH], f32)
            q_swpv = q_swp.rearrange("bt (g u h2) -> bt g u h2", g=6, u=2)
            nc.vector.tensor_copy(q_swpv[:, 0:4, 0, :], qp0v[:, :, 1, :])
            nc.vector.tensor_copy(q_swpv[:, 0:4, 1, :], qp0v[:, :, 0, :])
            nc.vector.tensor_copy(q_swpv[:, 4:6, 0, :], qp1v[:, 0:2, 1, :])
            nc.vector.tensor_copy(q_swpv[:, 4:6, 1, :], qp1v[:, 0:2, 0, :])
            k_sb = cpool.tile([BT, H], f32)
            nc.vector.tensor_copy(k_sb, qp1[:, 256:384])
            k_swp = cpool.tile([BT, H], f32)
            k_swpv = k_swp.rearrange("bt (u h2) -> bt u h2", u=2)
            nc.vector.tensor_copy(k_swpv[:, 0, :], qp1v[:, 2, 1, :])
            nc.vector.tensor_copy(k_swpv[:, 1, :], qp1v[:, 2, 0, :])

            # ---- q -> qT (f32), RoPE (scale folded in coeffs), cast bf16.
            # q column order is (lb, g, t) so per-batch slices are contiguous
            # (matmul stationary operands need single-free-dim APs).
            qTr = cpool.tile([128, BL, G, T], f32)
            qSw = cpool.tile([128, BL, G, T], f32)
            for g in range(G):
                tp = pt.tile([128, BT], f32, tag="pt")
                nc.tensor.transpose(tp, q_sb[:, g * H:(g + 1) * H], identf[:BT, :BT])
                nc.vector.tensor_copy(qTr[:, :, g, :],
                                      tp.rearrange("h (l t) -> h l t", l=BL))
                tps = pt.tile([128, BT], f32, tag="pt")
                nc.tensor.transpose(tps, q_swp[:, g * H:(g + 1) * H], identf[:BT, :BT])
                nc.vector.tensor_copy(qSw[:, :, g, :],
                                      tps.rearrange("h (l t) -> h l t", l=BL))
            qtmp = cpool.tile([128, BL, G, T], f32)
            qRf = cpool.tile([128, BL, G, T], f32)
            qR = cpool.tile([128, BL, G, T], bf16)
            nc.vector.tensor_tensor(qtmp, qTr, rq_t[:, 0], mybir.AluOpType.mult)
            nc.vector.tensor_tensor(qRf, qSw, rq_t[:, 1], mybir.AluOpType.mult)
            nc.vector.tensor_tensor(qR, qRf, qtmp, mybir.AluOpType.add)

            # ---- k_new -> kT (f32), RoPE (no scale), cast bf16
            kTr = cpool.tile([128, BT], f32)
            kSw = cpool.tile([128, BT], f32)
            tpk = pt.tile([128, BT], f32, tag="pt")
            nc.tensor.transpose(tpk, k_sb, identf[:BT, :BT])
            nc.vector.tensor_copy(kTr, tpk)
            tpks = pt.tile([128, BT], f32, tag="pt")
            nc.tensor.transpose(tpks, k_swp, identf[:BT, :BT])
            nc.vector.tensor_copy(kSw, tpks)
            ktmp = cpool.tile([128, BT], f32)
            kRf = cpool.tile([128, BT], f32)
            kR = cpool.tile([128, BT], bf16)
            nc.vector.tensor_tensor(ktmp, kTr, rk_t[:, 0], mybir.AluOpType.mult)
            nc.vector.tensor_tensor(kRf, kSw, rk_t[:, 1], mybir.AluOpType.mult)
            nc.vector.tensor_tensor(kR, kRf, ktmp, mybir.AluOpType.add)

            # ---- fused new-token block for both local batches:
            # logits [32 new rows, (lb, g, t) queries]; cross-batch pairs are
            # masked to NEG so exp() zeroes them.
            qRflat = qR.rearrange("h l g t -> h (l g t)")
            lpn = pl.tile([BT, BL, G, T], f32, tag="pl", name="lpn")
            nc.tensor.matmul(lpn, kR, qRflat, start=True, stop=True)
            nc.vector.tensor_tensor(lpn, lpn, nm_t, mybir.AluOpType.add)
            wTn = sp.tile([BT, BL * GT], bf16, tag="wtn")
            nc.scalar.activation(wTn, lpn, mybir.ActivationFunctionType.Exp)

            # ---- attention hot loop, software pipelined (depth 2):
            # issue order L(0) L(1) L(2) Q(0) L(3) Q(1) ... so the PE keeps
            # running logits while the scalar engine exponentiates.
            qkvPs = []
            qkvT = cpool.tile([128, G, BT], bf16)

            def issue_L(lb, gg, lps):
                for j in range(4):
                    nc.tensor.matmul(lps[:, j, :], kcts[lb][:, 4 * gg + j, :],
                                     qRflat[:, lb * GT:(lb + 1) * GT],
                                     start=True, stop=True)

            def issue_E(gg, lps):
                wT = wtp.tile([128, 4, GT], bf16, tag="wt")
                nc.scalar.activation(wT, lps, mybir.ActivationFunctionType.Exp)
                return wT

            def issue_Q(lb, gg, wT):
                for j in range(4):
                    nc.tensor.matmul(qkvPs[lb], wT[:, j, :], vcts[lb][:, 4 * gg + j, :],
                                     start=(gg == 0 and j == 0), stop=False,
                                     skip_group_check=True)

            def issue_normalize(lb):
                # normalize, cast bf16, transpose into qkvT columns
                qkvP = qkvPs[lb]
                rec = sp.tile([GT, 1], f32, tag="rec")
                nc.vector.reciprocal(rec, qkvP[:, H:H + 1])
                qkvN = sp.tile([GT, H], bf16, tag="qkvN")
                nc.vector.tensor_scalar_mul(qkvN, qkvP[:, 0:H], rec)
                tp3 = pt.tile([128, GT], bf16, tag="pt")
                nc.tensor.transpose(tp3, qkvN, identb)
                nc.vector.tensor_copy(
                    qkvT[:, :, lb * T:(lb + 1) * T],
                    tp3.rearrange("h (g t) -> h g t", g=G))

            for lb in range(BL):
                qkvPs.append(pacc.tile([GT, H + 1], f32, tag="pacc", name=f"qkvP{lb}"))

            for lb in range(BL):
                lps_q = []
                for gg in range(NG):
                    lps = pl.tile([128, 4, GT], f32, tag="pl")
                    issue_L(lb, gg, lps)
                    lps_q.append((gg, lps))
                    if len(lps_q) >= 3 or gg == NG - 1:
                        while (len(lps_q) >= 3) or (gg == NG - 1 and lps_q):
                            g0, l0 = lps_q.pop(0)
                            issue_Q(lb, g0, issue_E(g0, l0))
                    # overlap the previous batch's normalize with this stream
                    if lb == 1 and gg == 1:
                        issue_normalize(0)
                # new tokens (kv rows [cur, cur+T) live on-chip)
                nc.tensor.matmul(qkvPs[lb], wTn[:, lb * GT:(lb + 1) * GT], vN,
                                 start=False, stop=True, skip_group_check=True)
            issue_normalize(1)

            # ---- output projection: out[bt, d] = sum_g qkvT[h,g,bt]^T wo[h,g,d]
            for db in range(3):
                oP = pp.tile([BT, 512], f32, tag="pp")
                for g in range(G):
                    nc.tensor.matmul(oP, qkvT[:, g, :], wo_t[:, g, db * 512:(db + 1) * 512],
                                     start=(g == 0), stop=(g == G - 1))
                o_sb = sp.tile([BT, 512], f32, tag="osb")
                nc.vector.tensor_copy(o_sb, oP)
                nc.sync.dma_start(out=outp_d[:, db * 512:(db + 1) * 512], in_=o_sb)

    nc.compile()  # bacc passes: splits multi-wait instructions (TRN2 allows 1)
    return nc


# ---------------------------------------------------------------- entry point
def kernel(x, k_cache, v_cache, wq, bq, wk, bk, wv, bv, wo,
           segment_ids, start_ind, cur_ind):
    x = np.asarray(x, np.float32)
    k_cache = np.asarray(k_cache, np.float32)
    v_cache = np.asarray(v_cache, np.float32)
    wq = np.asarray(wq, np.float32)
    wk = np.asarray(wk, np.float32)
    wv = np.asarray(wv, np.float32)
    wo = np.asarray(wo, np.float32)
    cur = int(np.asarray(cur_ind))

    mask, positions = _host_mask(segment_ids, start_ind, cur)

    spec_ok = (
        cur % 512 == 0 and 0 < cur and cur + T <= S
        and not np.any(np.asarray(bq)) and not np.any(np.asarray(bk))
        and not np.any(np.asarray(bv))
        and not np.any(mask[:, :, cur + T:])          # nothing attended past new rows
        and bool(np.all(np.any(mask, axis=2)))        # no fully-masked query row
        and bool(np.all(mask[:, :, :cur]))            # all old-cache rows attended
    )
    if not spec_ok:
        return _numpy_reference(x, k_cache, v_cache, wq, bq, wk, bk, wv, bv, wo,
                                segment_ids, start_ind, cur)

    sold = cur
    key = sold
    if key not in _built:
        _built[key] = _build(sold)
    nc = _built[key]

    inputs = dict(x=x, k_cache=k_cache, v_cache=v_cache, wq=wq, wk=wk, wv=wv,
                  wo=wo, segment_ids=segment_ids, start_ind=start_ind,
                  cur_ind=cur)
    in_maps = _make_in_maps(inputs, sold, mask=mask, positions=positions)

    global _last_in_maps
    _last_in_maps = in_maps

    import os
    from concourse.bass_utils import run_bass_kernel_spmd
    trace = os.environ.get("KERNEL_TRACE", "0") == "1"
    res = run_bass_kernel_spmd(nc, in_maps, core_ids=list(range(8)), trace=trace)
    if trace and res.exec_time_ns is not None:
        print(f"HW exec time: {res.exec_time_ns} ns")

    out = np.zeros((B, T, D), np.float32)
    for c in range(8):
        bg = c // 2
        out[bg * BL:(bg + 1) * BL] += res.results[c]["out"].reshape(BL, T, D)
    return out


def _make_in_maps(inputs, sold, mask=None, positions=None):
    x = np.asarray(inputs["x"], np.float32)
    k_cache = np.asarray(inputs["k_cache"], np.float32)
    v_cache = np.asarray(inputs["v_cache"], np.float32)
    wq = np.asarray(inputs["wq"], np.float32)
    wk = np.asarray(inputs["wk"], np.float32)
    wv = np.asarray(inputs["wv"], np.float32)
    wo = np.asarray(inputs["wo"], np.float32)
    cur = int(np.asarray(inputs["cur_ind"]))
    if mask is None:
        mask, positions = _host_mask(inputs["segment_ids"], inputs["start_ind"], cur)

    sin, cos = _host_rope(positions)  # [b, t, 64]
    scale = np.float32(H ** -0.5)
    NJ = sold // 128
    BT = BL * T

    # rope coeff layouts: rows h<64 -> (cos, -sin); h>=64 -> (cos, +sin)
    def rope_pack(bsl, with_scale, with_g):
        # returns [128, 2, BL, G, T] (with_g) or [128, 2, BL*T]
        cs = cos[bsl]  # [BL, T, 64]
        sn = sin[bsl]
        ccol = np.transpose(cs, (2, 0, 1)).reshape(64, BL * T)  # [64, (b,t)]
        scol = np.transpose(sn, (2, 0, 1)).reshape(64, BL * T)
        c128 = np.concatenate([ccol, ccol], axis=0)     # [128, BT]
        s128 = np.concatenate([-scol, scol], axis=0)
        if with_scale:
            c128 = c128 * scale
            s128 = s128 * scale
        pack = np.stack([c128, s128], axis=1)           # [128, 2, BT]
        if with_g:  # broadcast over heads, columns ordered (lb, g, t)
            pack = np.repeat(pack.reshape(128, 2, BL, 1, T), G, axis=3)
        return np.ascontiguousarray(pack, np.float32)

    in_maps = []
    for c in range(8):
        k = c % 2
        bg = c // 2
        bsl = slice(bg * BL, (bg + 1) * BL)
        wq4 = wq.reshape(DC, 128, N, H)[:, :, k * G:(k + 1) * G, :].reshape(DC, 128, GH)
        wk4 = wk.reshape(DC, 128, K, H)[:, :, k, :]
        wv4 = wv.reshape(DC, 128, K, H)[:, :, k, :]
        # packed [wq | wk | wv], laid out [128, DC, 1024] so it loads in one DMA
        wpk = np.concatenate([wq4, wk4, wv4], axis=2).transpose(1, 0, 2).astype(BF)
        wot = wo[k * G:(k + 1) * G].transpose(1, 0, 2).astype(BF)  # [H, G, D]
        kc = k_cache[bsl, :sold, k, :]                             # [BL, sold, H]
        # ktp[lb, h, jj, p] = K[lb, p*NJ + jj, h]
        ktp = np.ascontiguousarray(
            kc.reshape(BL, 128, NJ, H).transpose(0, 3, 2, 1).astype(BF))
        vcs = np.concatenate(
            [v_cache[bsl, :sold, k, :], np.ones((BL, sold, 1), np.float32)],
            axis=-1).astype(BF)
        vcp = np.ascontiguousarray(vcs.reshape(BL, 128, NJ, H + 1))
        xr = np.ascontiguousarray(
            x[bsl].reshape(BT, DC, 128).transpose(2, 1, 0).astype(BF))
        # fused new-token additive mask [(lb_s, t_s), lb_q, g, t_q]:
        # cross-batch pairs NEG; within batch, reference mask for rows cur..cur+T
        m_new = mask[bsl][:, :, cur:cur + T]            # [lb_q, t_q, t_s]
        nm = np.full((BL, T, BL, G, T), NEG, np.float32)
        for lbq in range(BL):
            blk = np.where(m_new[lbq], np.float32(0), np.float32(NEG))  # [t_q, t_s]
            nm[lbq, :, lbq, :, :] = blk.T[:, None, :]   # [t_s, g, t_q]
        nm = nm.reshape(BT, BL, G, T)
        in_maps.append({
            "xt": xr,
            "wpk": np.ascontiguousarray(wpk),
            "wok": np.ascontiguousarray(wot),
            "ktp": ktp,
            "vcp": vcp,
            "ropeq": rope_pack(bsl, True, True),
            "ropek": rope_pack(bsl, False, False),
            "nmask": np.ascontiguousarray(nm, np.float32),
        })

    return in_maps
